# revision 1
# baseline (speedup 1.0000x reference)
"""Multi-head attention (no mask) Trainium2 kernel, SPMD over 8 NeuronCores.

Problem: x[2,2048,1024] @ wq/wk/wv[1024,1024] (+zero biases) -> 16-head
scaled-dot-product attention (softmax over full sequence, no causal mask),
output [2,2048,1024] fp32.

Sharding: batch x head-quad. Core i handles batch i//4 and heads
4*(i%4)..4*(i%4)+4 (256 output columns). Per-core inputs: x[b] [2048,1024],
w*[:, c0:c0+256], b*[c0:c0+256]; per-core output out[2048, 256]. Host
concatenates heads and stacks batches.

Design notes (the scalar engine is the hard floor: ~109us/core of pure exp):
  - everything in the attention inner loop runs in the PE's 64x128 tiling
    mode: score matmuls contract over head_dim=64, so the two heads of a
    pair ride concurrent row-tiles T0/T8; AV matmuls are split into 64-key
    parity halves, also T0/T8 pairs (order T0,T8,T8,T0 per K so the two
    psy accumulator banks never see overlapping write windows).
  - exp consumes a continuous stream of (K, h) score slots packed 3 per
    psum tile [128, 3, 512] (N=1536 per activation, the largest batch that
    still allows double buffering: 2x3 banks + 2 psy banks = 8).
  - softmax denominator rides the AV matmul: v65 col 64 is ones, psy row 64
    accumulates sum(exp); finalize PE-transposes psy, multiplies by the
    reciprocal, and DMAs out. Finalize + projection chunks borrow psum
    slots from the psy/pss pools at block boundaries.
  - x fp32 -> bf16 via SWDGE cast to a DRAM scratch (big contiguous
    descriptors), then HWDGE 2-byte transpose-DMA into xT.
"""

import os
import sys

import numpy as np

for _p in ("/opt/trn_rl_repo", "/root/.axon_site/_ro/trn_rl_repo"):
    if _p not in sys.path and os.path.isdir(_p):
        sys.path.append(_p)

from contextlib import ExitStack

import concourse.bass as bass
import concourse.tile as tile
from concourse import bacc, masks, mybir
from concourse.bass_utils import run_bass_kernel_spmd

FP32 = mybir.dt.float32
BF16 = mybir.dt.bfloat16
Exp = mybir.ActivationFunctionType.Exp

N_CORES = 8
B, S, D = 2, 2048, 1024
COLS = 256            # output columns per core = 4 heads x 64
HP = 2                # head pairs per core
HD = 64               # head dim
SCALE = 0.125         # 1 / sqrt(HD)
QCH = 512             # q chunk (psum free dim)
NKC = 16              # kc128 chunks
NQC = S // QCH        # 4
NJ = QCH // 128       # 4
DT = D // 128         # 8 contraction tiles for projections
GSLOT = 3             # score slots per psum tile / exp instruction

_CACHED_NC = None


def build_nc(reps=1):
    nc = bacc.Bacc("TRN2", target_bir_lowering=False, debug=False,
                   num_devices=N_CORES)

    x = nc.dram_tensor("x", [S, D], FP32, kind="ExternalInput").ap()
    w_ap = {}
    b_ap = {}
    for p in ("q", "k", "v"):
        w_ap[p] = nc.dram_tensor(f"w{p}", [D, COLS], FP32,
                                 kind="ExternalInput").ap()
        b_ap[p] = nc.dram_tensor(f"b{p}", [COLS], FP32,
                                 kind="ExternalInput").ap()
    out = nc.dram_tensor("out", [S, COLS], FP32, kind="ExternalOutput").ap()

    with tile.TileContext(nc) as tc, ExitStack() as ctx:
        dram_pool = ctx.enter_context(tc.tile_pool(name="dram", bufs=2,
                                                   space="DRAM"))
        const_pool = ctx.enter_context(tc.tile_pool(name="const", bufs=1))
        wst_pool = ctx.enter_context(tc.tile_pool(name="wst", bufs=2))
        w_pool = ctx.enter_context(tc.tile_pool(name="w", bufs=1))
        xt_pool = ctx.enter_context(tc.tile_pool(name="xt", bufs=1))
        qkv_pool = ctx.enter_context(tc.tile_pool(name="qkv", bufs=1))
        v65_pool = ctx.enter_context(tc.tile_pool(name="v65", bufs=1))
        att_pool = ctx.enter_context(tc.tile_pool(name="att", bufs=16))
        fin_pool = ctx.enter_context(tc.tile_pool(name="fin", bufs=4))
        yout_pool = ctx.enter_context(tc.tile_pool(name="yout", bufs=4))
        ps_s = ctx.enter_context(tc.tile_pool(name="pss", bufs=2,
                                              space="PSUM"))
        ps_y = ctx.enter_context(tc.tile_pool(name="psy", bufs=2,
                                              space="PSUM"))

        id_f32 = const_pool.tile([128, 128], FP32, tag="idf")
        id_bf16 = const_pool.tile([128, 128], BF16, tag="idb")
        masks.make_identity(nc, id_f32[:])
        masks.make_identity(nc, id_bf16[:])

        def emit_w(st, p):
            w32 = wst_pool.tile([128, DT, COLS], FP32, tag="wst",
                                name=f"w32{p}")
            nc.scalar.dma_start(out=w32[:],
                                in_=w_ap[p].rearrange("(t p) c -> p t c",
                                                      p=128))
            wt = w_pool.tile([128, DT, COLS], BF16, tag=f"w{p}")
            nc.vector.tensor_copy(wt[:], w32[:])
            st["w", p] = wt
            bt = w_pool.tile([128, HP], FP32, tag=f"b{p}")
            nc.sync.dma_start(out=bt[:],
                              in_=b_ap[p].rearrange("(hp c) -> c hp", c=128))
            st["b", p] = bt

        def emit_cast(st, sc):
            if "x16" not in st:
                st["x16"] = dram_pool.tile([S, D], BF16, name="x16")
            x16 = st["x16"]
            nc.gpsimd.dma_start(out=x16[sc * QCH:(sc + 1) * QCH, :],
                                in_=x[sc * QCH:(sc + 1) * QCH, :])

        def emit_xt(st, sc):
            if "xt" not in st:
                st["xt"] = xt_pool.tile([128, DT, S], BF16, tag="xt",
                                        name="xt")
            xt = st["xt"]
            x16v = st["x16"].rearrange("s (t p) -> s t p", p=128)
            for t in range(DT):
                nc.sync.dma_start(
                    out=xt[:, t, sc * QCH:(sc + 1) * QCH],
                    in_=x16v[sc * QCH:(sc + 1) * QCH, t, :], transpose=True)

        def emit_proj(st, hp, p, sc):
            if (hp, p) not in st:
                st[hp, p] = qkv_pool.tile([128, S], BF16, tag=f"{p}T{hp}",
                                          name=f"{p}T{hp}")
            pt = st[hp, p]
            xt = st["xt"]
            wt = st["w", p]
            ps = ps_s.tile([128, QCH], FP32, tag="s", name="psproj")
            for t in range(DT):
                nc.tensor.matmul(
                    ps[:], lhsT=wt[:, t, hp * 128:(hp + 1) * 128],
                    rhs=xt[:, t, sc * QCH:(sc + 1) * QCH],
                    start=(t == 0), stop=(t == DT - 1))
            nc.vector.tensor_scalar_add(
                pt[:, sc * QCH:(sc + 1) * QCH], ps[:],
                st["b", p][:, hp:hp + 1])

        def emit_v65(st, hp, quad):
            if (hp, "v65") not in st:
                v65 = v65_pool.tile([128, NKC, 2, 65], BF16, tag=f"v65{hp}",
                                    name=f"v65{hp}")
                nc.vector.memset(v65[:, :, :, 64], 1.0)
                st[hp, "v65"] = v65
            v65 = st[hp, "v65"]
            vT = st[hp, "v"]
            for K in range(quad * 4, quad * 4 + 4):
                pvt = ps_s.tile([128, 2, 1024], BF16, tag="s", name="psvt")
                for h in range(2):
                    nc.tensor.transpose(
                        pvt[:, h, 0:64],
                        vT[h * HD:(h + 1) * HD, K * 128:(K + 1) * 128],
                        id_bf16[h * HD:(h + 1) * HD, h * HD:(h + 1) * HD])
                for h in range(2):
                    nc.vector.tensor_copy(v65[:, K, h, 0:64], pvt[:, h, 0:64])

        def emit_finalize_h(hp, qc, h, psy, yo):
            # merge the two parity banks, transpose, normalize into yo
            ysb = fin_pool.tile([65, QCH], FP32, tag="ysb", name="ysb")
            nc.vector.tensor_copy(ysb[:], psy[0][:])
            nc.vector.scalar_tensor_tensor(
                ysb[:], psy[1][:], 0.0, ysb[:],
                mybir.AluOpType.add, mybir.AluOpType.add)
            for jj in range(NJ):
                pyt = ps_y.tile([128, 65], FP32, tag="y", name="psyt")
                nc.tensor.transpose(pyt[:],
                                    ysb[:, jj * 128:(jj + 1) * 128],
                                    id_f32[0:65, 0:65])
                rc = fin_pool.tile([128, 1], FP32, tag="rc", name="rc")
                nc.vector.reciprocal(rc[:], pyt[:, 64:65])
                nc.vector.tensor_scalar_mul(
                    yo[:, jj, h * HD:(h + 1) * HD], pyt[:, 0:64], rc[:])
            if h == 1:
                nc.sync.dma_start(
                    out=out[qc * QCH:(qc + 1) * QCH,
                            hp * 128:(hp + 1) * 128].rearrange(
                        "(j p) c -> p j c", p=128),
                    in_=yo[:])

        def emit_prologue(st):
            for p in ("q", "k", "v"):
                emit_w(st, p)
            for sc in range(NQC):
                emit_cast(st, sc)
            for sc in range(NQC):
                emit_xt(st, sc)
            emit_proj(st, 0, "k", 0)
            emit_proj(st, 0, "q", 0)
            emit_proj(st, 0, "v", 0)
            emit_proj(st, 0, "k", 1)
            emit_proj(st, 0, "v", 1)
            emit_v65(st, 0, 0)
            emit_v65(st, 0, 1)

        def prologue_dma_items(st):
            # DMA-only work for the next rep: safe to emit early (no PE-queue
            # slot waits; transfers just queue behind this rep's DMAs)
            items = []
            for p in ("q", "k", "v"):
                items.append(lambda p=p: emit_w(st, p))
            for sc in range(NQC):
                items.append(lambda sc=sc: emit_cast(st, sc))
            for sc in range(NQC):
                items.append(lambda sc=sc: emit_xt(st, sc))
            return items

        def prologue_pe_items(st):
            # PE work for the next rep: its qkv/v65 hp0 tile slots free only
            # once this rep's hp0 phases are done, and the PE queue is
            # in-order, so these must be emitted during this rep's hp1 half.
            return [
                lambda: emit_proj(st, 0, "k", 0),
                lambda: emit_proj(st, 0, "q", 0),
                lambda: emit_proj(st, 0, "v", 0),
                lambda: emit_proj(st, 0, "k", 1),
                lambda: emit_proj(st, 0, "v", 1),
                lambda: emit_v65(st, 0, 0),
                lambda: emit_v65(st, 0, 1),
            ]

        def trickle_items(st):
            return [
                lambda: emit_proj(st, 0, "k", 2),
                lambda: emit_proj(st, 0, "v", 2),
                lambda: emit_v65(st, 0, 2),
                lambda: emit_proj(st, 0, "k", 3),
                lambda: emit_proj(st, 0, "v", 3),
                lambda: emit_v65(st, 0, 3),
                lambda: emit_proj(st, 0, "q", 1),
                lambda: emit_proj(st, 1, "k", 0),
                lambda: emit_proj(st, 1, "v", 0),
                lambda: emit_v65(st, 1, 0),
                lambda: emit_proj(st, 1, "k", 1),
                lambda: emit_proj(st, 1, "v", 1),
                lambda: emit_v65(st, 1, 1),
                lambda: emit_proj(st, 1, "q", 0),
                lambda: emit_proj(st, 1, "k", 2),
                lambda: emit_proj(st, 1, "v", 2),
                lambda: emit_v65(st, 1, 2),
                lambda: emit_proj(st, 1, "k", 3),
                lambda: emit_proj(st, 1, "v", 3),
                lambda: emit_v65(st, 1, 3),
                lambda: emit_proj(st, 0, "q", 2),
                lambda: emit_proj(st, 1, "q", 1),
                lambda: emit_proj(st, 0, "q", 3),
                lambda: emit_proj(st, 1, "q", 2),
                lambda: emit_proj(st, 1, "q", 3),
            ]

        def run_rep(state, pending):
            pending = list(pending)
            pending.reverse()

            def hook():
                if pending:
                    item = pending.pop()
                    if item is not None:
                        item()

            # --- slot stream -------------------------------------------------
            # slots iterate (hp, qc, K, h) with h innermost so score matmuls
            # alternate PE row-tiles T0/T8 (concurrent pairs). AV runs in
            # h-phases: h0's AVs go out as soon as att is ready; h1's are
            # deferred (att tiles held in SBUF) until h0's two parity psum
            # banks are finalized, so each psy bank is only ever written by
            # one row-tile (T0 -> par0 bank, T8 -> par1 bank).
            stream = [(hp, qc, K, h)
                      for hp in range(HP) for qc in range(NQC)
                      for K in range(NKC) for h in range(2)]

            att_of = {}
            cur = {"tile": None, "at_slots": []}
            phase = {"key": None, "psy": None, "fin": None}
            blockstate = {}

            def get_bs(hp, qc):
                bs = blockstate.get((hp, qc))
                if bs is None:
                    bs = {"h0_done": False, "deferred": [],
                          "yo": yout_pool.tile([128, NJ, 128], FP32,
                                               tag="yo", name="yo")}
                    blockstate[(hp, qc)] = bs
                return bs

            def do_av(hp, qc, h, K):
                if phase["key"] != (hp, qc, h):
                    if phase["fin"] is not None:
                        phase["fin"]()
                    phase["key"] = (hp, qc, h)
                    phase["psy"] = [ps_y.tile([65, QCH], FP32, tag="y",
                                              name=f"psy{par}")
                                    for par in range(2)]
                    psy_now = phase["psy"]
                    yo_now = get_bs(hp, qc)["yo"]
                    phase["fin"] = (lambda hp=hp, qc=qc, h=h, psy=psy_now,
                                    yo=yo_now:
                                    emit_finalize_h(hp, qc, h, psy, yo))
                psy = phase["psy"]
                v65 = state[hp, "v65"]
                a, i = att_of.pop((hp, qc, h, K))
                for par in range(2):
                    nc.tensor.matmul(
                        psy[par][:],
                        lhsT=v65[par * 64:(par + 1) * 64, K, h, :],
                        rhs=a[par * 64:(par + 1) * 64, i, :],
                        start=(K == 0), stop=(K == NKC - 1))

            def flush():
                n = len(cur["at_slots"])
                if n == 0:
                    return
                at = att_pool.tile([128, GSLOT, QCH], BF16, tag="att",
                                   name="att")
                nc.scalar.activation(at[:, 0:n, :], cur["tile"][:, 0:n, :],
                                     Exp, scale=SCALE)
                slots = cur["at_slots"]
                cur["tile"] = None
                cur["at_slots"] = []
                for idx, (hp, qc, K, h) in enumerate(slots):
                    att_of[(hp, qc, h, K)] = (at, idx)
                    bs = get_bs(hp, qc)
                    if h == 0:
                        do_av(hp, qc, 0, K)
                        hook()
                        if K == NKC - 1:
                            bs["h0_done"] = True
                            for K1 in bs["deferred"]:
                                do_av(hp, qc, 1, K1)
                                hook()
                            bs["deferred"] = []
                    else:
                        if bs["h0_done"]:
                            do_av(hp, qc, 1, K)
                            hook()
                        else:
                            bs["deferred"].append(K)

            for slot in stream:
                hp, qc, K, h = slot
                if cur["tile"] is None:
                    cur["tile"] = ps_s.tile([128, GSLOT, QCH], FP32, tag="s",
                                            name="pss")
                idx = len(cur["at_slots"])
                nc.tensor.matmul(
                    cur["tile"][:, idx, :],
                    lhsT=state[hp, "k"][h * HD:(h + 1) * HD,
                                        K * 128:(K + 1) * 128],
                    rhs=state[hp, "q"][h * HD:(h + 1) * HD,
                                       qc * QCH:(qc + 1) * QCH],
                    start=True, stop=True)
                cur["at_slots"].append(slot)
                if len(cur["at_slots"]) == GSLOT:
                    flush()
            flush()
            while pending:
                pending.pop()()
            phase["fin"]()

        sts = [{} for _ in range(reps)]
        emit_prologue(sts[0])
        for r in range(reps):
            pend = trickle_items(sts[r])
            if r + 1 < reps:
                pend += prologue_dma_items(sts[r + 1])
                pend += [None] * (140 - len(pend))
                pend += prologue_pe_items(sts[r + 1])
            run_rep(sts[r], pend)

    nc.compile()
    return nc


def get_nc():
    global _CACHED_NC
    if _CACHED_NC is None:
        _CACHED_NC = build_nc()
    return _CACHED_NC


def make_in_maps(x, wq, bq, wk, bk, wv, bv):
    in_maps = []
    for i in range(N_CORES):
        b = i // 4
        c0 = (i % 4) * COLS
        in_maps.append({
            "x": np.ascontiguousarray(x[b], dtype=np.float32),
            "wq": np.ascontiguousarray(wq[:, c0:c0 + COLS], dtype=np.float32),
            "wk": np.ascontiguousarray(wk[:, c0:c0 + COLS], dtype=np.float32),
            "wv": np.ascontiguousarray(wv[:, c0:c0 + COLS], dtype=np.float32),
            "bq": np.ascontiguousarray(bq[c0:c0 + COLS], dtype=np.float32),
            "bk": np.ascontiguousarray(bk[c0:c0 + COLS], dtype=np.float32),
            "bv": np.ascontiguousarray(bv[c0:c0 + COLS], dtype=np.float32),
        })
    return in_maps


def assemble(res, inputs=None):
    batches = []
    for b in range(B):
        parts = [res.results[b * 4 + q]["out"] for q in range(4)]
        batches.append(np.concatenate(parts, axis=1))
    return np.stack(batches).astype(np.float32)


def kernel(x, wq, bq, wk, bk, wv, bv):
    nc = get_nc()
    in_maps = make_in_maps(x, wq, bq, wk, bk, wv, bv)
    res = run_bass_kernel_spmd(nc, in_maps, list(range(N_CORES)))
    out = assemble(res)
    kernel.last_results = res
    return out



# revision 5
# speedup vs baseline: 6.6967x; 6.6967x over previous
"""Multi-head attention (no mask) Trainium2 kernel, SPMD over 8 NeuronCores.

Problem: x[2,2048,1024] @ wq/wk/wv[1024,1024] (+zero biases) -> 16-head
scaled-dot-product attention (softmax over full sequence, no causal mask),
output [2,2048,1024] fp32.

Sharding: batch x head-quad. Core i handles batch i//4 and heads
4*(i%4)..4*(i%4)+4 (256 output columns). Host concatenates heads and
stacks batches.

v2 design (ACT-exp is the hard floor: ~110us/core):
  - fp16 datapath everywhere below the fp32 psum accumulators.
  - scores: row-paired (h0 rows 0-63 / h1 rows 64-127) 64-contraction
    matmuls into alternating 3-slot/2-slot psum tiles (6KB+4KB), exp'd by
    ACT in N=1536/1024 batches.
  - AV: col-tiled pairs - h0 -> psy[0:64], h1 -> psy[64:128] - full
    128-key contraction, both heads concurrent, accumulated over 16 key
    chunks in one psum bank.
  - softmax denominator: DVE accumulates exp'd slots into dacc[128,2h,512]
    (4x fp16 mode); per phase one col-tiled pair of ones-matmuls
    broadcast-reduces dacc into psd[128,512]; DVE reciprocal + multiply
    normalizes psy; transpose-DMA + SWDGE-cast write the output.
  - PE FIFO discipline: AVs of group g are emitted after the scores of
    group g+1, so the in-order PE queue never blocks ACT's score supply.
  - V reaches [key, dim] layout via one transpose-DMA per head pair;
    x/w reach fp16 via SWDGE DRAM casts + transpose-DMA (x) / direct (w).
"""

import os
import sys

import numpy as np

for _p in ("/opt/trn_rl_repo", "/root/.axon_site/_ro/trn_rl_repo"):
    if _p not in sys.path and os.path.isdir(_p):
        sys.path.append(_p)

from contextlib import ExitStack

import concourse.bass as bass
import concourse.tile as tile
from concourse import bacc, mybir
from concourse.bass_utils import run_bass_kernel_spmd

FP32 = mybir.dt.float32
FP16 = mybir.dt.float16
Exp = mybir.ActivationFunctionType.Exp
Add = mybir.AluOpType.add
Mult = mybir.AluOpType.mult

N_CORES = 8
B, S, D = 2, 2048, 1024
COLS = 256            # output columns per core = 4 heads x 64
HP = 2                # head pairs per core
HD = 64               # head dim
SCALE = 0.125         # 1 / sqrt(HD)
QCH = 512             # q chunk (psum free dim)
NKC = 16              # 128-key chunks
NQC = S // QCH        # 4
DT = D // 128         # 8 contraction tiles for projections
NPH = HP * NQC        # 8 phases
SLOTS_PER_PH = NKC * 2

_CACHED_NC = None


def build_nc(reps=1):
    nc = bacc.Bacc("TRN2", target_bir_lowering=False, debug=False,
                   num_devices=N_CORES)

    x = nc.dram_tensor("x", [S, D], FP32, kind="ExternalInput").ap()
    w_ap = {}
    b_ap = {}
    for p in ("q", "k", "v"):
        w_ap[p] = nc.dram_tensor(f"w{p}", [D, COLS], FP32,
                                 kind="ExternalInput").ap()
        b_ap[p] = nc.dram_tensor(f"b{p}", [COLS], FP32,
                                 kind="ExternalInput").ap()
    out = nc.dram_tensor("out", [S, COLS], FP32, kind="ExternalOutput").ap()

    with tile.TileContext(nc) as tc, ExitStack() as ctx:
        dram_pool = ctx.enter_context(tc.tile_pool(name="dram", bufs=2,
                                                   space="DRAM"))
        const_pool = ctx.enter_context(tc.tile_pool(name="const", bufs=1))
        w_pool = ctx.enter_context(tc.tile_pool(name="w", bufs=2))
        xt_pool = ctx.enter_context(tc.tile_pool(name="xt", bufs=2))
        qkv_pool = ctx.enter_context(tc.tile_pool(name="qkv", bufs=1))
        v64_pool = ctx.enter_context(tc.tile_pool(name="v64", bufs=1))
        att_pool = ctx.enter_context(tc.tile_pool(name="att", bufs=6))
        dacc_pool = ctx.enter_context(tc.tile_pool(name="dacc", bufs=2))
        fin_pool = ctx.enter_context(tc.tile_pool(name="fin", bufs=2))
        ps_s = ctx.enter_context(tc.tile_pool(name="pss", bufs=1,
                                              space="PSUM"))
        ps_y = ctx.enter_context(tc.tile_pool(name="psy", bufs=1,
                                              space="PSUM"))

        ones = const_pool.tile([128, HD], FP16, tag="ones")
        nc.vector.memset(ones[:], 1.0)

        # ---- prologue emitters -------------------------------------------
        def emit_w_dma(st, p):
            # fp32 DRAM -> fp16 DRAM via SWDGE cast, then straight to SBUF
            w16 = dram_pool.tile([D, COLS], FP16, tag=f"w16{p}",
                                 name=f"w16{p}")
            nc.gpsimd.dma_start(out=w16[:], in_=w_ap[p])
            wt = w_pool.tile([128, DT, COLS], FP16, tag=f"w{p}",
                             name=f"wt{p}")
            nc.sync.dma_start(out=wt[:],
                              in_=w16.rearrange("(t p) c -> p t c", p=128))
            st["w", p] = wt
            bt = w_pool.tile([128, HP], FP32, tag=f"b{p}", name=f"bt{p}")
            nc.sync.dma_start(out=bt[:],
                              in_=b_ap[p].rearrange("(hp c) -> c hp", c=128))
            st["b", p] = bt

        def emit_xcast(st, sc):
            if "x16" not in st:
                st["x16"] = dram_pool.tile([S, D], FP16, tag="x16",
                                           name="x16")
            nc.gpsimd.dma_start(out=st["x16"][sc * QCH:(sc + 1) * QCH, :],
                                in_=x[sc * QCH:(sc + 1) * QCH, :])

        def emit_xt(st, t):
            if "xt" not in st:
                st["xt"] = xt_pool.tile([128, DT, S], FP16, tag="xt",
                                        name="xt")
            nc.sync.dma_start(out=st["xt"][:, t, :],
                              in_=st["x16"][:, t * 128:(t + 1) * 128],
                              transpose=True)

        # ---- projections --------------------------------------------------
        def emit_proj_mms(st, hp, p, sc, t0, t1):
            key = (hp, p, "ps", sc)
            if key not in st:
                st[key] = ps_s.tile([128, QCH], FP32, tag="pj", bufs=1,
                                    name="pspj")
            ps = st[key]
            xt = st["xt"]
            wt = st["w", p]
            for t in range(t0, t1):
                nc.tensor.matmul(
                    ps[:], lhsT=wt[:, t, hp * 128:(hp + 1) * 128],
                    rhs=xt[:, t, sc * QCH:(sc + 1) * QCH],
                    start=(t == 0), stop=(t == DT - 1))

        def emit_proj_copy(st, hp, p, sc):
            if (hp, p) not in st:
                st[hp, p] = qkv_pool.tile([128, S], FP16, tag=f"{p}T{hp}",
                                          name=f"{p}T{hp}")
            ps = st.pop((hp, p, "ps", sc))
            nc.vector.tensor_scalar_add(
                st[hp, p][:, sc * QCH:(sc + 1) * QCH], ps[:],
                st["b", p][:, hp:hp + 1])

        def emit_v64(st, hp):
            # vT[hp] [128(2h x 64d), 2048k] -> v64 [128k, 16K, 128d2]
            v64 = v64_pool.tile([128, NKC, 128], FP16, tag=f"v64{hp}",
                                name=f"v64{hp}")
            nc.sync.dma_start(out=v64[:], in_=st[hp, "v"][:], transpose=True)
            st[hp, "v64"] = v64

        def proj_items(st, hp, p, sc):
            return [
                lambda: emit_proj_mms(st, hp, p, sc, 0, 4),
                lambda: (emit_proj_mms(st, hp, p, sc, 4, 8),
                         emit_proj_copy(st, hp, p, sc)),
            ]

        def prologue_dma_items(st):
            items = []
            for p in ("q", "k", "v"):
                items.append(lambda p=p: emit_w_dma(st, p))
            for sc in range(NQC):
                items.append(lambda sc=sc: emit_xcast(st, sc))
            for t in range(DT):
                items.append(lambda t=t: emit_xt(st, t))
            return items

        def prologue_pe_items(st):
            # everything the first phase (hp0, qc0) needs: k all-sc (keys
            # sweep the full sequence), q sc0, v all-sc + v64.
            items = []
            for sc in range(NQC):
                items += proj_items(st, 0, "k", sc)
            items += proj_items(st, 0, "q", 0)
            for sc in range(NQC):
                items += proj_items(st, 0, "v", sc)
            items.append(lambda: emit_v64(st, 0))
            return items

        def own_trickle_items(st):
            items = []
            for sc in range(1, NQC):
                items += proj_items(st, 0, "q", sc)
            for sc in range(NQC):
                items += proj_items(st, 1, "k", sc)
            for sc in range(NQC):
                items += proj_items(st, 1, "v", sc)
            items.append(lambda: emit_v64(st, 1))
            for sc in range(NQC):
                items += proj_items(st, 1, "q", sc)
            return items

        # ---- finalize -----------------------------------------------------
        def emit_finalize(st, hp, qc, psy, dacc):
            psd = ps_y.tile([128, QCH], FP32, tag="d", bufs=1, name="psd")
            for h in range(2):
                nc.tensor.matmul(
                    psd[h * HD:(h + 1) * HD, :], lhsT=ones[:],
                    rhs=dacc[:, h, :], start=True, stop=True,
                    tile_position=(0, h * HD))
            rp = fin_pool.tile([128, QCH], FP16, tag="rp", name="rp")
            y16 = fin_pool.tile([128, QCH], FP16, tag="y16", name="y16")
            with nc.allow_low_precision(reason="fp16 softmax normalize"):
                nc.vector.reciprocal(rp[:], psd[:])
                nc.vector.tensor_tensor(y16[:], psy[:], rp[:], Mult)
            yo = fin_pool.tile([128, NQC, 128], FP16, tag="yo", name="yo")
            nc.sync.dma_start(out=yo[:], in_=y16[:], transpose=True)
            nc.gpsimd.dma_start(
                out=out[qc * QCH:(qc + 1) * QCH,
                        hp * 128:(hp + 1) * 128].rearrange(
                    "(j p) c -> p j c", p=128),
                in_=yo[:])

        # ---- the attention stream ----------------------------------------
        def run_rep(st, own_items, late_items, late_start):
            own_items = list(own_items)[::-1]
            late_items = list(late_items)[::-1]

            slots = [(hp, qc, K, h)
                     for hp in range(HP) for qc in range(NQC)
                     for K in range(NKC) for h in range(2)]
            ns = len(slots)
            groups = []
            i = 0
            while i < ns:
                sz = 3 if len(groups) % 2 == 0 else 2
                sz = min(sz, ns - i)
                groups.append((i, sz))
                i += sz

            att_of = {}          # global slot idx -> (att_tile, idx_in_tile)
            av_ready = []        # global slot idxs whose att is available
            av_deferred = []     # held back to keep h-pairs adjacent
            phase_av_count = {}
            phase_psy = {}
            phase_dacc = {}
            phase_first = {}     # (ph, h) -> True until first dacc write

            def get_dacc(ph):
                if ph not in phase_dacc:
                    phase_dacc[ph] = dacc_pool.tile([128, 2, QCH], FP16,
                                                    tag="dacc", name="dacc")
                    phase_first[ph, 0] = True
                    phase_first[ph, 1] = True
                return phase_dacc[ph]

            def get_psy(ph):
                # only called from emit_av: phase ph's first AV is emitted
                # strictly after the previous phase's finalize, so the
                # bufs=1 rotation sees all prior readers already emitted.
                if ph not in phase_psy:
                    phase_psy[ph] = ps_y.tile([128, QCH], FP32, tag="y",
                                              bufs=1, name="psy")
                    phase_av_count[ph] = 0
                return phase_psy[ph]

            def emit_av(si):
                hp, qc, K, h = slots[si]
                ph = hp * NQC + qc
                psy = get_psy(ph)
                at, idx = att_of.pop(si)
                nc.tensor.matmul(
                    psy[h * HD:(h + 1) * HD, :],
                    lhsT=st[hp, "v64"][:, K, h * HD:(h + 1) * HD],
                    rhs=at[:, idx, :],
                    start=(K == 0), stop=(K == NKC - 1),
                    tile_position=(0, h * HD))
                phase_av_count[ph] += 1
                if phase_av_count[ph] == SLOTS_PER_PH:
                    emit_finalize(st, hp, qc, phase_psy.pop(ph),
                                  phase_dacc.pop(ph))

            def flush_avs(final=False):
                pend = av_deferred + av_ready
                av_ready.clear()
                av_deferred.clear()
                while len(pend) >= 2:
                    emit_av(pend.pop(0))
                    emit_av(pend.pop(0))
                if pend:
                    if final:
                        emit_av(pend.pop(0))
                    else:
                        av_deferred.extend(pend)

            def emit_dacc(gstart, gsize, at):
                # batch adds into dacc[:, h, :]; h == global idx parity.
                i = 0
                while i < gsize:
                    si = gstart + i
                    hp, qc, K, h = slots[si]
                    ph = hp * NQC + qc
                    dacc = get_dacc(ph)
                    # run of 2 with h==0 first, staying inside the phase
                    run2 = (h == 0 and i + 1 < gsize
                            and slots[si + 1][:2] == (hp, qc))
                    with nc.allow_low_precision(reason="fp16 denom accum"):
                        if run2:
                            dst = dacc[:, 0:2, :]
                            src = at[:, i:i + 2, :]
                            if phase_first[ph, 0] or phase_first[ph, 1]:
                                nc.vector.tensor_copy(dst, src)
                            else:
                                nc.vector.tensor_tensor(dst, dst, src, Add)
                            phase_first[ph, 0] = False
                            phase_first[ph, 1] = False
                            i += 2
                        else:
                            dst = dacc[:, h, :]
                            src = at[:, i, :]
                            if phase_first[ph, h]:
                                nc.vector.tensor_copy(dst, src)
                                phase_first[ph, h] = False
                            else:
                                nc.vector.tensor_tensor(dst, dst, src, Add)
                            i += 1

            for gi, (gstart, gsize) in enumerate(groups):
                tag = "s3" if gsize == 3 else "s2"
                width = 3 if gsize == 3 else 2
                pss = ps_s.tile([128, width, QCH], FP32, tag=tag, bufs=1,
                                name="pss")
                for i in range(gsize):
                    hp, qc, K, h = slots[gstart + i]
                    nc.tensor.matmul(
                        pss[:, i, :],
                        lhsT=st[hp, "k"][h * HD:(h + 1) * HD,
                                         K * 128:(K + 1) * 128],
                        rhs=st[hp, "q"][h * HD:(h + 1) * HD,
                                        qc * QCH:(qc + 1) * QCH],
                        start=True, stop=True)
                at = att_pool.tile([128, 3, QCH], FP16, tag="att",
                                   name="att")
                with nc.allow_low_precision(reason="fp16 attention"):
                    nc.scalar.activation(at[:, 0:gsize, :],
                                         pss[:, 0:gsize, :], Exp,
                                         scale=SCALE)
                for i in range(gsize):
                    att_of[gstart + i] = (at, i)

                # dacc adds must be emitted before this group's AVs can
                # trigger a finalize (the bcast-MM reads dacc), and AVs of
                # group g are only flushed in iteration g+1 so the PE FIFO
                # never blocks ahead of the next score group.
                emit_dacc(gstart, gsize, at)

                # trickle (always-ready PE work) before the AV batch
                if own_items:
                    own_items.pop()()
                if gi >= late_start and late_items:
                    late_items.pop()()

                flush_avs()
                av_ready.extend(range(gstart, gstart + gsize))

            flush_avs(final=True)
            while own_items:
                own_items.pop()()
            while late_items:
                late_items.pop()()

        # ---- rep loop -----------------------------------------------------
        sts = [{} for _ in range(reps)]
        for it in prologue_dma_items(sts[0]):
            it()
        for it in prologue_pe_items(sts[0]):
            it()
        for r in range(reps):
            late = []
            if r + 1 < reps:
                late = prologue_dma_items(sts[r + 1]) \
                    + prologue_pe_items(sts[r + 1])
            run_rep(sts[r], own_trickle_items(sts[r]), late,
                    late_start=55)

    nc.compile()
    return nc


def get_nc():
    global _CACHED_NC
    if _CACHED_NC is None:
        _CACHED_NC = build_nc()
    return _CACHED_NC


def make_in_maps(x, wq, bq, wk, bk, wv, bv):
    in_maps = []
    for i in range(N_CORES):
        b = i // 4
        c0 = (i % 4) * COLS
        in_maps.append({
            "x": np.ascontiguousarray(x[b], dtype=np.float32),
            "wq": np.ascontiguousarray(wq[:, c0:c0 + COLS], dtype=np.float32),
            "wk": np.ascontiguousarray(wk[:, c0:c0 + COLS], dtype=np.float32),
            "wv": np.ascontiguousarray(wv[:, c0:c0 + COLS], dtype=np.float32),
            "bq": np.ascontiguousarray(bq[c0:c0 + COLS], dtype=np.float32),
            "bk": np.ascontiguousarray(bk[c0:c0 + COLS], dtype=np.float32),
            "bv": np.ascontiguousarray(bv[c0:c0 + COLS], dtype=np.float32),
        })
    return in_maps


def assemble(res, inputs=None):
    batches = []
    for b in range(B):
        parts = [res.results[b * 4 + q]["out"] for q in range(4)]
        batches.append(np.concatenate(parts, axis=1))
    return np.stack(batches).astype(np.float32)


def kernel(x, wq, bq, wk, bk, wv, bv):
    nc = get_nc()
    in_maps = make_in_maps(x, wq, bq, wk, bk, wv, bv)
    res = run_bass_kernel_spmd(nc, in_maps, list(range(N_CORES)))
    out = assemble(res)
    kernel.last_results = res
    return out


# revision 11
# speedup vs baseline: 6.9153x; 1.0326x over previous
"""Multi-head attention (no mask) Trainium2 kernel, SPMD over 8 NeuronCores.

Problem: x[2,2048,1024] @ wq/wk/wv[1024,1024] (+zero biases) -> 16-head
scaled-dot-product attention (softmax over full sequence, no causal mask),
output [2,2048,1024] fp32.

Sharding: batch x head-quad. Core i handles batch i//4 and heads
4*(i%4)..4*(i%4)+4 (256 output columns). Host concatenates heads and
stacks batches.

v2 design (ACT-exp is the hard floor: ~110us/core):
  - fp16 datapath everywhere below the fp32 psum accumulators.
  - scores: row-paired (h0 rows 0-63 / h1 rows 64-127) 64-contraction
    matmuls into alternating 3-slot/2-slot psum tiles (6KB+4KB), exp'd by
    ACT in N=1536/1024 batches.
  - AV: col-tiled pairs - h0 -> psy[0:64], h1 -> psy[64:128] - full
    128-key contraction, both heads concurrent, accumulated over 16 key
    chunks in one psum bank.
  - softmax denominator: DVE accumulates exp'd slots into dacc[128,2h,512]
    (4x fp16 mode); per phase one col-tiled pair of ones-matmuls
    broadcast-reduces dacc into psd[128,512]; DVE reciprocal + multiply
    normalizes psy; transpose-DMA + SWDGE-cast write the output.
  - PE FIFO discipline: AVs of group g are emitted after the scores of
    group g+1, so the in-order PE queue never blocks ACT's score supply.
  - V reaches [key, dim] layout via one transpose-DMA per head pair;
    x/w reach fp16 via SWDGE DRAM casts + transpose-DMA (x) / direct (w).
"""

import os
import sys

import numpy as np

for _p in ("/opt/trn_rl_repo", "/root/.axon_site/_ro/trn_rl_repo"):
    if _p not in sys.path and os.path.isdir(_p):
        sys.path.append(_p)

from contextlib import ExitStack

import concourse.bass as bass
import concourse.tile as tile
from concourse import bacc, mybir
from concourse.bass_utils import run_bass_kernel_spmd

FP32 = mybir.dt.float32
FP16 = mybir.dt.float16
Exp = mybir.ActivationFunctionType.Exp
Add = mybir.AluOpType.add
Mult = mybir.AluOpType.mult

N_CORES = 8
B, S, D = 2, 2048, 1024
COLS = 256            # output columns per core = 4 heads x 64
HP = 2                # head pairs per core
HD = 64               # head dim
SCALE = 0.125         # 1 / sqrt(HD)
QCH = 512             # q chunk (psum free dim)
NKC = 16              # 128-key chunks
NQC = S // QCH        # 4
DT = D // 128         # 8 contraction tiles for projections
NPH = HP * NQC        # 8 phases
SLOTS_PER_PH = NKC * 2

_CACHED_NC = None


def build_nc(reps=1):
    nc = bacc.Bacc("TRN2", target_bir_lowering=False, debug=False,
                   num_devices=N_CORES)

    x = nc.dram_tensor("x", [S, D], FP32, kind="ExternalInput").ap()
    w_ap = {}
    b_ap = {}
    for p in ("q", "k", "v"):
        w_ap[p] = nc.dram_tensor(f"w{p}", [D, COLS], FP32,
                                 kind="ExternalInput").ap()
        b_ap[p] = nc.dram_tensor(f"b{p}", [COLS], FP32,
                                 kind="ExternalInput").ap()
    out = nc.dram_tensor("out", [S, COLS], FP32, kind="ExternalOutput").ap()

    with tile.TileContext(nc) as tc, ExitStack() as ctx:
        dram_pool = ctx.enter_context(tc.tile_pool(name="dram", bufs=2,
                                                   space="DRAM"))
        const_pool = ctx.enter_context(tc.tile_pool(name="const", bufs=1))
        w_pool = ctx.enter_context(tc.tile_pool(name="w", bufs=2))
        xt_pool = ctx.enter_context(tc.tile_pool(name="xt", bufs=2))
        qkv_pool = ctx.enter_context(tc.tile_pool(name="qkv", bufs=1))
        v64_pool = ctx.enter_context(tc.tile_pool(name="v64", bufs=1))
        att_pool = ctx.enter_context(tc.tile_pool(name="att", bufs=6))
        dacc_pool = ctx.enter_context(tc.tile_pool(name="dacc", bufs=2))
        fin_pool = ctx.enter_context(tc.tile_pool(name="fin", bufs=2))
        ps_s = ctx.enter_context(tc.tile_pool(name="pss", bufs=1,
                                              space="PSUM"))
        ps_y = ctx.enter_context(tc.tile_pool(name="psy", bufs=1,
                                              space="PSUM"))

        ones = const_pool.tile([128, HD], FP16, tag="ones")
        nc.vector.memset(ones[:], 1.0)

        # ---- prologue emitters -------------------------------------------
        def emit_w_dma(st, p):
            # fp32 DRAM -> fp16 DRAM via SWDGE cast, then straight to SBUF
            w16 = dram_pool.tile([D, COLS], FP16, tag=f"w16{p}",
                                 name=f"w16{p}")
            nc.gpsimd.dma_start(out=w16.rearrange("a b -> (a b)"),
                                in_=w_ap[p].rearrange("a b -> (a b)"))
            wt = w_pool.tile([128, DT, COLS], FP16, tag=f"w{p}",
                             name=f"wt{p}")
            nc.sync.dma_start(out=wt[:],
                              in_=w16.rearrange("(t p) c -> p t c", p=128))
            st["w", p] = wt
            bt = w_pool.tile([128, HP], FP32, tag=f"b{p}", name=f"bt{p}")
            nc.sync.dma_start(out=bt[:],
                              in_=b_ap[p].rearrange("(hp c) -> c hp", c=128))
            st["b", p] = bt

        def emit_xcast(st, sc):
            if "x16" not in st:
                st["x16"] = dram_pool.tile([S, D], FP16, tag="x16",
                                           name="x16")
            nc.gpsimd.dma_start(
                out=st["x16"][sc * QCH:(sc + 1) * QCH, :].rearrange(
                    "a b -> (a b)"),
                in_=x[sc * QCH:(sc + 1) * QCH, :].rearrange("a b -> (a b)"))

        def emit_xt(st, t):
            if "xt" not in st:
                st["xt"] = xt_pool.tile([128, DT, S], FP16, tag="xt",
                                        name="xt")
            nc.sync.dma_start(out=st["xt"][:, t, :],
                              in_=st["x16"][:, t * 128:(t + 1) * 128],
                              transpose=True)

        # ---- projections --------------------------------------------------
        def emit_proj_mms(st, hp, p, sc, t0, t1):
            key = (hp, p, "ps", sc)
            if key not in st:
                # shares the psd bank (tag "d"): projections and the
                # per-phase denominator reduce alternate through it
                st[key] = ps_y.tile([128, QCH], FP32, tag="d", bufs=1,
                                    name="pspj")
            ps = st[key]
            xt = st["xt"]
            wt = st["w", p]
            for t in range(t0, t1):
                nc.tensor.matmul(
                    ps[:], lhsT=wt[:, t, hp * 128:(hp + 1) * 128],
                    rhs=xt[:, t, sc * QCH:(sc + 1) * QCH],
                    start=(t == 0), stop=(t == DT - 1))

        def emit_proj_copy(st, hp, p, sc):
            if (hp, p) not in st:
                st[hp, p] = qkv_pool.tile([128, S], FP16, tag=f"{p}T{hp}",
                                          name=f"{p}T{hp}")
            ps = st.pop((hp, p, "ps", sc))
            nc.vector.tensor_scalar_add(
                st[hp, p][:, sc * QCH:(sc + 1) * QCH], ps[:],
                st["b", p][:, hp:hp + 1])

        def emit_v64(st, hp):
            # vT[hp] [128(2h x 64d), 2048k] -> v64 [128k, 16K, 128d2]
            v64 = v64_pool.tile([128, NKC, 128], FP16, tag=f"v64{hp}",
                                name=f"v64{hp}")
            nc.sync.dma_start(out=v64[:], in_=st[hp, "v"][:], transpose=True)
            st[hp, "v64"] = v64

        def proj_items(st, hp, p, sc):
            return [
                lambda: emit_proj_mms(st, hp, p, sc, 0, 4),
                lambda: (emit_proj_mms(st, hp, p, sc, 4, 8),
                         emit_proj_copy(st, hp, p, sc)),
            ]

        def prologue_dma_items(st):
            items = []
            for p in ("q", "k", "v"):
                items.append(lambda p=p: emit_w_dma(st, p))
            for sc in range(NQC):
                items.append(lambda sc=sc: emit_xcast(st, sc))
            for t in range(DT):
                items.append(lambda t=t: emit_xt(st, t))
            return items

        def prologue_pe_items(st):
            # everything the first phase (hp0, qc0) needs: k all-sc (keys
            # sweep the full sequence), q sc0, v all-sc + v64.
            items = []
            for sc in range(NQC):
                items += proj_items(st, 0, "k", sc)
            items += proj_items(st, 0, "q", 0)
            for sc in range(NQC):
                items += proj_items(st, 0, "v", sc)
            items.append(lambda: emit_v64(st, 0))
            return items

        def own_trickle_items(st):
            items = []
            for sc in range(1, NQC):
                items += proj_items(st, 0, "q", sc)
            for sc in range(NQC):
                items += proj_items(st, 1, "k", sc)
            for sc in range(NQC):
                items += proj_items(st, 1, "v", sc)
            items.append(lambda: emit_v64(st, 1))
            for sc in range(NQC):
                items += proj_items(st, 1, "q", sc)
            return items

        # ---- finalize -----------------------------------------------------
        def emit_finalize(st, hp, qc, psy, dacc):
            psd = ps_y.tile([128, QCH], FP32, tag="d", bufs=1, name="psd")
            for h in range(2):
                nc.tensor.matmul(
                    psd[h * HD:(h + 1) * HD, :], lhsT=ones[:],
                    rhs=dacc[:, h, :], start=True, stop=True,
                    tile_position=(0, h * HD))
            rp = fin_pool.tile([128, QCH], FP16, tag="rp", name="rp")
            y16 = fin_pool.tile([128, QCH], FP16, tag="y16", name="y16")
            with nc.allow_low_precision(reason="fp16 softmax normalize"):
                nc.vector.reciprocal(rp[:], psd[:])
                nc.vector.tensor_tensor(y16[:], psy[:], rp[:], Mult)
            yo = fin_pool.tile([128, NQC, 128], FP16, tag="yo", name="yo")
            nc.sync.dma_start(out=yo[:], in_=y16[:], transpose=True)
            nc.gpsimd.dma_start(
                out=out[qc * QCH:(qc + 1) * QCH,
                        hp * 128:(hp + 1) * 128].rearrange(
                    "(j p) c -> p j c", p=128),
                in_=yo[:])

        # ---- the attention stream ----------------------------------------
        def run_rep(st, own_items, late_items, late_start):
            own_items = list(own_items)[::-1]
            late_items = list(late_items)[::-1]

            slots = [(hp, qc, K, h)
                     for hp in range(HP) for qc in range(NQC)
                     for K in range(NKC) for h in range(2)]
            ns = len(slots)
            groups = []
            i = 0
            while i < ns:
                sz = min(3, ns - i)
                groups.append((i, sz))
                i += sz

            att_of = {}          # global slot idx -> (att_tile, idx_in_tile)
            av_ready = []        # global slot idxs whose att is available
            av_deferred = []     # held back to keep h-pairs adjacent
            phase_av_count = {}
            phase_psy = {}
            phase_dacc = {}
            phase_first = {}     # (ph, h) -> True until first dacc write

            def get_dacc(ph):
                if ph not in phase_dacc:
                    phase_dacc[ph] = dacc_pool.tile([128, 2, QCH], FP16,
                                                    tag="dacc", name="dacc")
                    phase_first[ph, 0] = True
                    phase_first[ph, 1] = True
                return phase_dacc[ph]

            def get_psy(ph):
                # only called from emit_av: phase ph's first AV is emitted
                # strictly after the previous phase's finalize, so the
                # bufs=1 rotation sees all prior readers already emitted.
                if ph not in phase_psy:
                    phase_psy[ph] = ps_y.tile([128, QCH], FP32, tag="y",
                                              bufs=1, name="psy")
                    phase_av_count[ph] = 0
                return phase_psy[ph]

            def emit_av(si):
                hp, qc, K, h = slots[si]
                ph = hp * NQC + qc
                psy = get_psy(ph)
                at, idx = att_of.pop(si)
                nc.tensor.matmul(
                    psy[h * HD:(h + 1) * HD, :],
                    lhsT=st[hp, "v64"][:, K, h * HD:(h + 1) * HD],
                    rhs=at[:, idx, :],
                    start=(K == 0), stop=(K == NKC - 1),
                    tile_position=(0, h * HD))
                phase_av_count[ph] += 1
                if phase_av_count[ph] == SLOTS_PER_PH:
                    emit_finalize(st, hp, qc, phase_psy.pop(ph),
                                  phase_dacc.pop(ph))

            def flush_avs(final=False):
                pend = av_deferred + av_ready
                av_ready.clear()
                av_deferred.clear()
                while len(pend) >= 2:
                    emit_av(pend.pop(0))
                    emit_av(pend.pop(0))
                if pend:
                    if final:
                        emit_av(pend.pop(0))
                    else:
                        av_deferred.extend(pend)

            def emit_dacc(gstart, gsize, at):
                # batch adds into dacc[:, h, :]; h == global idx parity.
                i = 0
                while i < gsize:
                    si = gstart + i
                    hp, qc, K, h = slots[si]
                    ph = hp * NQC + qc
                    dacc = get_dacc(ph)
                    # run of 2 with h==0 first, staying inside the phase
                    run2 = (h == 0 and i + 1 < gsize
                            and slots[si + 1][:2] == (hp, qc))
                    with nc.allow_low_precision(reason="fp16 denom accum"):
                        if run2:
                            dst = dacc[:, 0:2, :]
                            src = at[:, i:i + 2, :]
                            if phase_first[ph, 0] or phase_first[ph, 1]:
                                nc.vector.tensor_copy(dst, src)
                            else:
                                nc.vector.tensor_tensor(dst, dst, src, Add)
                            phase_first[ph, 0] = False
                            phase_first[ph, 1] = False
                            i += 2
                        else:
                            dst = dacc[:, h, :]
                            src = at[:, i, :]
                            if phase_first[ph, h]:
                                nc.vector.tensor_copy(dst, src)
                                phase_first[ph, h] = False
                            else:
                                nc.vector.tensor_tensor(dst, dst, src, Add)
                            i += 1

            for gi, (gstart, gsize) in enumerate(groups):
                tag = "sA" if gi % 2 == 0 else "sB"
                pss = ps_s.tile([128, 3, QCH], FP32, tag=tag, bufs=1,
                                name="pss")
                for i in range(gsize):
                    hp, qc, K, h = slots[gstart + i]
                    nc.tensor.matmul(
                        pss[:, i, :],
                        lhsT=st[hp, "k"][h * HD:(h + 1) * HD,
                                         K * 128:(K + 1) * 128],
                        rhs=st[hp, "q"][h * HD:(h + 1) * HD,
                                        qc * QCH:(qc + 1) * QCH],
                        start=True, stop=True)
                at = att_pool.tile([128, 3, QCH], FP16, tag="att",
                                   name="att")
                with nc.allow_low_precision(reason="fp16 attention"):
                    nc.scalar.activation(at[:, 0:gsize, :],
                                         pss[:, 0:gsize, :], Exp,
                                         scale=SCALE)
                for i in range(gsize):
                    att_of[gstart + i] = (at, i)

                # dacc adds must be emitted before this group's AVs can
                # trigger a finalize (the bcast-MM reads dacc), and AVs of
                # group g are only flushed in iteration g+1 so the PE FIFO
                # never blocks ahead of the next score group.
                emit_dacc(gstart, gsize, at)

                # trickle (always-ready PE work) before the AV batch
                if own_items:
                    own_items.pop()()
                if gi >= late_start and late_items:
                    late_items.pop()()

                flush_avs()
                av_ready.extend(range(gstart, gstart + gsize))

            flush_avs(final=True)
            while own_items:
                own_items.pop()()
            while late_items:
                late_items.pop()()

        # ---- rep loop -----------------------------------------------------
        sts = [{} for _ in range(reps)]
        for it in prologue_dma_items(sts[0]):
            it()
        for it in prologue_pe_items(sts[0]):
            it()
        for r in range(reps):
            late = []
            if r + 1 < reps:
                late = prologue_dma_items(sts[r + 1]) \
                    + prologue_pe_items(sts[r + 1])
            run_rep(sts[r], own_trickle_items(sts[r]), late,
                    late_start=50)

    nc.compile()
    return nc


def get_nc():
    global _CACHED_NC
    if _CACHED_NC is None:
        _CACHED_NC = build_nc()
    return _CACHED_NC


def make_in_maps(x, wq, bq, wk, bk, wv, bv):
    in_maps = []
    for i in range(N_CORES):
        b = i // 4
        c0 = (i % 4) * COLS
        in_maps.append({
            "x": np.ascontiguousarray(x[b], dtype=np.float32),
            "wq": np.ascontiguousarray(wq[:, c0:c0 + COLS], dtype=np.float32),
            "wk": np.ascontiguousarray(wk[:, c0:c0 + COLS], dtype=np.float32),
            "wv": np.ascontiguousarray(wv[:, c0:c0 + COLS], dtype=np.float32),
            "bq": np.ascontiguousarray(bq[c0:c0 + COLS], dtype=np.float32),
            "bk": np.ascontiguousarray(bk[c0:c0 + COLS], dtype=np.float32),
            "bv": np.ascontiguousarray(bv[c0:c0 + COLS], dtype=np.float32),
        })
    return in_maps


def assemble(res, inputs=None):
    batches = []
    for b in range(B):
        parts = [res.results[b * 4 + q]["out"] for q in range(4)]
        batches.append(np.concatenate(parts, axis=1))
    return np.stack(batches).astype(np.float32)


def kernel(x, wq, bq, wk, bk, wv, bv):
    nc = get_nc()
    in_maps = make_in_maps(x, wq, bq, wk, bk, wv, bv)
    res = run_bass_kernel_spmd(nc, in_maps, list(range(N_CORES)))
    out = assemble(res)
    kernel.last_results = res
    return out
